# revision 1
# baseline (speedup 1.0000x reference)
# Trainium2 Bass kernel for nn_MixedFFN (B=8, T=2048, D=1024, F=4096, LNS=32).
#
# Sharding across 8 NeuronCores (no collectives needed):
#   - Shared-FFN branch (tokens 0..2015): core c handles batch row c.
#   - Per-position branch (last 32 token positions, distinct weights per
#     position): core c handles positions 4c..4c+3 for all 8 batch rows.
#
# Per-core math:
#   shared: ysT = (gelu(x_c @ W1S) @ W2S).T computed as
#     L1: hT[f, t] = sum_k W1S[k, f] * xT[k, t]      (lhsT = W1S tile, rhs = xT)
#         a = gelu(h) stored transposed [F, tokens] in fp16
#     L2: ysT[d, t] = sum_f W2S[f, d] * aT[f, t]     (lhsT = W2S tile fp16, rhs = aT)
#   per-position q: h = x_pos @ W1NS[q] (M=8 batch rows), gelu, transpose on PE,
#     y = a @ W2NS[q] via lhsT = aT[f, b] and rhs = W2NS[q] tiles.
#
# dtypes: all matmuls in fp16 (4-byte operands stream at half PE rate)
# (inputs/weights cast on host; activations cast by the gelu activation op).
# PSUM accumulates in fp32 everywhere, outputs are fp32.

import numpy as np

B, T, D, F, LNS = 8, 2048, 1024, 4096, 32
S = T - LNS            # 2016 shared tokens per batch row
NCORES = 8
QPC = LNS // NCORES    # 4 positions per core
TPAD = 2048            # shared tokens padded to multiple of 1024
NBLK = 2               # token blocks for the shared branch
BLK = TPAD // NBLK     # 1024 tokens per block
KD = D // 128          # 8  k-tiles over D
MF = F // 128          # 32 m-tiles over F
W1G = 16               # W1S column groups (2 m-tiles = 256 cols each)
W1GW = F // W1G        # 256
NQ = 8                 # D-column tiles for L2 output (dq)
PPN1 = 16              # per-position L1 chunks over F (256 wide)
PPW1 = F // PPN1       # 256
PPN2 = 2               # per-position L2 chunks over D (512 wide)
PPW2 = D // PPN2       # 512

_CACHE = {}


def _build_nc(loop_n=0, parts="all"):
    """Build + bacc-compile the single-core SPMD program. Cached per process.

    loop_n > 0 wraps the whole body in a hardware For_i loop that repeats the
    kernel loop_n times inside one NEFF execution — a timing instrument only.
    parts: "all" | "shared" | "pp" — emit only a subset (timing experiments).
    """
    key = ("nc", loop_n, parts)
    if key in _CACHE:
        return _CACHE[key]

    import concourse.mybir as mybir
    import concourse.tile as tile
    from concourse import bacc
    from concourse.masks import make_identity

    f32 = mybir.dt.float32
    f32r = mybir.dt.float32r
    f16 = mybir.dt.float16
    GELU = mybir.ActivationFunctionType.Gelu

    nc = bacc.Bacc(None, target_bir_lowering=False)

    # ---- kernel I/O (per-core shapes; host packs these layouts) ----
    xt_d = nc.dram_tensor("xt", [128, KD, TPAD], f16, kind="ExternalInput")
    w1_d = nc.dram_tensor("w1", [W1G, 128, KD, W1GW], f16, kind="ExternalInput")
    w2_d = nc.dram_tensor("w2", [NQ, 128, MF, 128], f16, kind="ExternalInput")
    w1n_d = nc.dram_tensor("w1n", [QPC, 128, PPN1, KD, PPW1], f16, kind="ExternalInput")
    w2n_d = nc.dram_tensor("w2n", [QPC, 128, PPN2, MF, PPW2], f16, kind="ExternalInput")
    xp_d = nc.dram_tensor("xp", [128, KD, B * QPC], f16, kind="ExternalInput")
    yst_d = nc.dram_tensor("yst", [D, TPAD], f32, kind="ExternalOutput")
    ytp_d = nc.dram_tensor("ytp", [B * QPC, D], f32, kind="ExternalOutput")

    with tile.TileContext(nc) as tc:
        with (
            tc.tile_pool(name="constp", bufs=1) as constp,
            tc.tile_pool(name="xqp", bufs=2) as xqp,        # [128, 8, 512] f32r, 2MB
            tc.tile_pool(name="w1p", bufs=2) as w1p,        # [128, 8, 256] f32r, 1MB
            tc.tile_pool(name="atp", bufs=1) as atp,        # [128, 32, 1024] f16, 8.4MB
            tc.tile_pool(name="w2p", bufs=2) as w2p,        # [128, 32, 128] f16, 1MB
            tc.tile_pool(name="w1np", bufs=2) as w1np,      # [128, 8, 256] f32r, 1MB
            tc.tile_pool(name="w2np", bufs=2) as w2np,      # [128, 8, 512] f16, 1MB
            tc.tile_pool(name="xpp", bufs=1) as xpp,
            tc.tile_pool(name="appp", bufs=2) as appp,      # [8, 4096] f16
            tc.tile_pool(name="atppp", bufs=2) as atppp,    # [128, 32, 8] f16
            tc.tile_pool(name="outp", bufs=3) as outp,      # [128, 512] f32
            tc.tile_pool(name="ypp", bufs=2) as ypp,        # [8, 512] f32
            tc.tile_pool(name="psum", bufs=2, space="PSUM") as psum,
        ):
            import contextlib

            identity = constp.tile([128, 128], f16)
            make_identity(nc, identity)

            loop_cm = (
                tc.For_i(0, loop_n, 1, hint_engines=(mybir.EngineType.PE,))
                if loop_n
                else contextlib.nullcontext()
            )
            with loop_cm:
              xp_sb = xpp.tile([128, KD, B * QPC], f16)
              nc.sync.dma_start(out=xp_sb[:], in_=xp_d[:])

              # ---------- emission units ----------
              # Shared L1 for one block: produce aT fp16 [128, MF, BLK]
              def shared_l1_block(blk):
                  at_b = atp.tile([128, MF, BLK], f16, name=f"at_{blk}", tag="at")
                  xq = [None, None]
                  for tch in range(2):
                      xq[tch] = xqp.tile([128, KD, 512], f16, name=f"xq_{blk}_{tch}", tag="xq")
                      nc.sync.dma_start(
                          out=xq[tch][:],
                          in_=xt_d[:, :, blk * BLK + tch * 512 : blk * BLK + (tch + 1) * 512],
                      )

                  def unit(g):
                      w1g = w1p.tile([128, KD, W1GW], f16, name=f"w1g_{blk}_{g}", tag="w1g")
                      nc.sync.dma_start(out=w1g[:], in_=w1_d[g])
                      for ml in range(2):
                          m = 2 * g + ml
                          for tch in range(2):
                              ps = psum.tile([128, 512], f32, tag="l1", bufs=2, name=f"psl1_{blk}_{m}_{tch}")
                              for k in range(KD):
                                  nc.tensor.matmul(
                                      ps[:],
                                      w1g[:, k, ml * 128 : (ml + 1) * 128],
                                      xq[tch][:, k, :],
                                      start=(k == 0),
                                      stop=(k == KD - 1),
                                  )
                              nc.scalar.activation(
                                  at_b[:, m, tch * 512 : (tch + 1) * 512], ps[:], GELU
                              )

                  return at_b, [lambda g=g: unit(g) for g in range(W1G)]

              # Shared L2 for one block, one dq column: ysT tile [128, 512] x2
              def shared_l2_unit(blk, at_b, dq):
                  w2g = w2p.tile([128, MF, 128], f16, name=f"w2g_{blk}_{dq}", tag="w2g")
                  nc.sync.dma_start(out=w2g[:], in_=w2_d[dq])
                  for tch in range(2):
                      ps = psum.tile([128, 512], f32, tag="l2", bufs=2, name=f"psl2_{blk}_{dq}_{tch}")
                      for k in range(MF):
                          nc.tensor.matmul(
                              ps[:],
                              w2g[:, k, :],
                              at_b[:, k, tch * 512 : (tch + 1) * 512],
                              start=(k == 0),
                              stop=(k == MF - 1),
                          )
                      ot = outp.tile([128, 512], f32, name=f"ot_{blk}_{dq}_{tch}", tag="ot")
                      nc.vector.tensor_copy(ot[:], ps[:])
                      nc.sync.dma_start(
                          out=yst_d[
                              dq * 128 : (dq + 1) * 128,
                              blk * BLK + tch * 512 : blk * BLK + (tch + 1) * 512,
                          ],
                          in_=ot[:],
                      )

              # Per-position L1 chunk: h chunk [8, 256] -> gelu into app_q
              def pp_l1_unit(q, app_q, nch):
                  w1t = w1np.tile([128, KD, PPW1], f16, name=f"w1n_{q}_{nch}", tag="w1n")
                  nc.sync.dma_start(out=w1t[:], in_=w1n_d[q, :, nch])
                  ps = psum.tile([8, 512], f32, tag="pp", bufs=2, name=f"pspp1_{q}_{nch}")
                  pss = ps[:, :PPW1]
                  for k in range(KD):
                      nc.tensor.matmul(
                          pss,
                          xp_sb[:, k, q * B : (q + 1) * B],
                          w1t[:, k, :],
                          start=(k == 0),
                          stop=(k == KD - 1),
                      )
                  nc.scalar.activation(
                      app_q[:, nch * PPW1 : (nch + 1) * PPW1], pss, GELU
                  )

              # Per-position transposes: app_q [8, 4096] -> atpp_q [128, 32, 8]
              def pp_transpose_unit(q, app_q, atpp_q, f):
                  tp = psum.tile([128, 8], f16, tag="tp", bufs=2, name=f"pst_{q}_{f}")
                  nc.tensor.transpose(
                      tp[:], app_q[:, f * 128 : (f + 1) * 128], identity[:8, :8]
                  )
                  nc.vector.tensor_copy(atpp_q[:, f, :], tp[:])

              # Per-position L2 for (q, nch): y chunk [8, 512]
              def pp_l2_unit(q, atpp_q, nch):
                  ps = psum.tile([8, 512], f32, tag="pp", bufs=2, name=f"pspp2_{q}_{nch}")
                  for kg in range(4):
                      w2t = w2np.tile([128, 8, PPW2], f16, name=f"w2n_{q}_{nch}_{kg}", tag="w2n")
                      nc.sync.dma_start(
                          out=w2t[:], in_=w2n_d[q, :, nch, kg * 8 : (kg + 1) * 8, :]
                      )
                      for kk in range(8):
                          k = kg * 8 + kk
                          nc.tensor.matmul(
                              ps[:],
                              atpp_q[:, k, :],
                              w2t[:, kk, :],
                              start=(k == 0),
                              stop=(k == MF - 1),
                          )
                  yt = ypp.tile([8, 512], f32, name=f"yt_{q}_{nch}", tag="yt")
                  nc.vector.tensor_copy(yt[:], ps[:])
                  nc.sync.dma_start(
                      out=ytp_d[q * B : (q + 1) * B, nch * PPW2 : (nch + 1) * PPW2],
                      in_=yt[:],
                  )

              # ---------- emission schedule ----------
              # Side work (per-position branch) is interleaved between shared
              # units so its big weight DMAs overlap the shared-branch compute.
              side = []
              app_tiles = {}
              atpp_tiles = {}
              for q in range(QPC):
                  def mk_app(q=q):
                      app_tiles[q] = appp.tile([8, F], f16, name=f"app_{q}", tag="app")
                  side.append(mk_app)
                  for nch in range(PPN1):
                      side.append(lambda q=q, nch=nch: pp_l1_unit(q, app_tiles[q], nch))
                  def mk_atpp(q=q):
                      atpp_tiles[q] = atppp.tile([128, MF, 8], f16, name=f"atpp_{q}", tag="atpp")
                  side.append(mk_atpp)
                  for f in range(MF):
                      side.append(lambda q=q, f=f: pp_transpose_unit(q, app_tiles[q], atpp_tiles[q], f))
                  for nch in range(PPN2):
                      side.append(lambda q=q, nch=nch: pp_l2_unit(q, atpp_tiles[q], nch))

              side_i = 0

              def emit_side(n):
                  nonlocal side_i
                  budget = n
                  while side_i < len(side) and budget > 0:
                      side[side_i]()
                      side_i += 1
                      budget -= 1

              # Per shared unit, emit a few side units. Side list length:
              # 4 * (1 + 16 + 1 + 32 + 2) = 208 units (transposes are tiny).
              # Shared units: 2 blocks * (16 L1 + 8 L2) = 48.
              for blk in range(NBLK):
                  at_b, l1_units = shared_l1_block(blk)
                  for g, u in enumerate(l1_units):
                      u()
                      emit_side(4)
                  for dq in range(NQ):
                      shared_l2_unit(blk, at_b, dq)
                      emit_side(5)
              emit_side(len(side))

    nc.compile()
    _CACHE[key] = nc
    return nc


def pack_inputs(x, W1S, W2S, W1NS, W2NS):
    """Build the 8 per-core input maps (numpy, host-side layout packing)."""
    x = np.asarray(x, dtype=np.float32)
    W1S = np.asarray(W1S, dtype=np.float32)
    W2S = np.asarray(W2S, dtype=np.float32)
    W1NS = np.asarray(W1NS, dtype=np.float32)
    W2NS = np.asarray(W2NS, dtype=np.float32)

    # Shared weights: identical on every core.
    w1_pk = np.ascontiguousarray(
        W1S.reshape(KD, 128, W1G, W1GW).transpose(2, 1, 0, 3).astype(np.float16)
    )
    w2_pk = np.ascontiguousarray(
        W2S.reshape(MF, 128, NQ, 128).transpose(2, 1, 0, 3).astype(np.float16)
    )

    in_maps = []
    for c in range(NCORES):
        xs = np.zeros((TPAD, D), dtype=np.float32)
        xs[:S] = x[c, :S]
        xt = np.ascontiguousarray(
            xs.T.reshape(KD, 128, TPAD).transpose(1, 0, 2).astype(np.float16)
        )

        w1n = np.ascontiguousarray(
            W1NS[QPC * c : QPC * (c + 1)]
            .reshape(QPC, KD, 128, PPN1, PPW1)
            .transpose(0, 2, 3, 1, 4)
            .astype(np.float16)
        )
        w2n = np.ascontiguousarray(
            W2NS[QPC * c : QPC * (c + 1)]
            .reshape(QPC, MF, 128, PPN2, PPW2)
            .transpose(0, 2, 3, 1, 4)
            .astype(np.float16)
        )
        xpos = x[:, S + QPC * c : S + QPC * (c + 1), :]          # [B, QPC, D]
        xp = np.ascontiguousarray(
            xpos.transpose(2, 1, 0)
            .reshape(KD, 128, QPC * B)
            .transpose(1, 0, 2)
            .astype(np.float16)
        )
        in_maps.append(
            {"xt": xt, "w1": w1_pk, "w2": w2_pk, "w1n": w1n, "w2n": w2n, "xp": xp}
        )
    return in_maps


def unpack_outputs(results):
    """Assemble the full [B, T, D] output from the 8 per-core result maps."""
    out = np.empty((B, T, D), dtype=np.float32)
    for c in range(NCORES):
        yst = results[c]["yst"]          # [D, TPAD]
        ytp = results[c]["ytp"]          # [B*QPC, D]
        out[c, :S, :] = yst[:, :S].T
        for q in range(QPC):
            out[:, S + QPC * c + q, :] = ytp[q * B : (q + 1) * B, :]
    return out


def kernel(x, W1S, W2S, W1NS, W2NS):
    from concourse.bass_utils import run_bass_kernel_spmd

    nc = _build_nc()
    in_maps = pack_inputs(x, W1S, W2S, W1NS, W2NS)
    res = run_bass_kernel_spmd(nc, in_maps, core_ids=list(range(NCORES)))
    return unpack_outputs(res.results)



# revision 2
# speedup vs baseline: 139.7544x; 139.7544x over previous
# Trainium2 Bass kernel for nn_MixedFFN (B=8, T=2048, D=1024, F=4096, LNS=32).
#
# Sharding across 8 NeuronCores (no collectives needed):
#   - Shared-FFN branch (tokens 0..2015): core c handles batch row c.
#   - Per-position branch (last 32 token positions, distinct weights per
#     position): core c handles positions 4c..4c+3 for all 8 batch rows.
#
# v2 changes vs baseline:
#   - Per-position matmuls are column-tiled: the 4 positions' 8-token
#     stationary operands sit in PE column groups 0..3 (tile_position=(0,32q))
#     and their weight streams run concurrently -> ~4x PP matmul throughput.
#   - PP gelu activations run on the full [128,512] PSUM tile (1 op, not 4).
#   - PP transposes batched: one [128,128] PE transpose covers all 4
#     positions (32 transposes total instead of 128 + 128 copies).
#   - PP weight DMAs are 2 MB contiguous-per-partition transfers.
#
# dtypes: all matmuls in fp16; PSUM accumulates fp32; outputs fp32.

import numpy as np

B, T, D, F, LNS = 8, 2048, 1024, 4096, 32
S = T - LNS            # 2016 shared tokens per batch row
NCORES = 8
QPC = LNS // NCORES    # 4 positions per core
TPAD = 2048            # shared tokens padded to multiple of 1024
NBLK = 2               # token blocks for the shared branch
BLK = TPAD // NBLK     # 1024 tokens per block
KD = D // 128          # 8  k-tiles over D
MF = F // 128          # 32 m-tiles over F
W1G = 16               # W1S column groups (2 m-tiles = 256 cols each)
W1GW = F // W1G        # 256
NQ = 8                 # D-column tiles for L2 output (dq)
PPN1 = 8               # per-position L1 chunks over F (512 wide)
PPW1 = F // PPN1       # 512
PPKH = 2               # k-halves for pp L1 weight DMA (4 k-tiles each)
PPN2 = 2               # per-position L2 chunks over D (512 wide)
PPW2 = D // PPN2       # 512
PPKG = 8               # k-groups for pp L2 weight DMA (4 k-tiles each)

_CACHE = {}


def _build_nc(loop_n=0, parts="all"):
    """Build + bacc-compile the single-core SPMD program. Cached per process.

    loop_n > 0 wraps the whole body in a hardware For_i loop that repeats the
    kernel loop_n times inside one NEFF execution — a timing instrument only.
    parts: "all" | "shared" | "pp" — emit only a subset (timing experiments).
    """
    key = ("nc", loop_n, parts)
    if key in _CACHE:
        return _CACHE[key]

    import concourse.mybir as mybir
    import concourse.tile as tile
    from concourse import bacc
    from concourse.masks import make_identity

    f32 = mybir.dt.float32
    f16 = mybir.dt.float16
    GELU = mybir.ActivationFunctionType.Gelu

    nc = bacc.Bacc(None, target_bir_lowering=False)

    # ---- kernel I/O (per-core shapes; host packs these layouts) ----
    xt_d = nc.dram_tensor("xt", [128, KD, TPAD], f16, kind="ExternalInput")
    w1_d = nc.dram_tensor("w1", [W1G, 128, KD, W1GW], f16, kind="ExternalInput")
    w2_d = nc.dram_tensor("w2", [NQ, 128, MF, 128], f16, kind="ExternalInput")
    w1n_d = nc.dram_tensor(
        "w1n", [PPN1, PPKH, 128, QPC, KD // PPKH, PPW1], f16, kind="ExternalInput"
    )
    w2n_d = nc.dram_tensor(
        "w2n", [PPN2, PPKG, 128, QPC, MF // PPKG, PPW2], f16, kind="ExternalInput"
    )
    xp_d = nc.dram_tensor("xp", [128, KD, QPC * B], f16, kind="ExternalInput")
    yst_d = nc.dram_tensor("yst", [D, TPAD], f32, kind="ExternalOutput")
    ytp_d = nc.dram_tensor("ytp", [QPC * B, D], f32, kind="ExternalOutput")

    with tile.TileContext(nc) as tc:
        with (
            tc.tile_pool(name="constp", bufs=1) as constp,
            tc.tile_pool(name="xqp", bufs=2) as xqp,        # [128, 8, 512] f16, 1MB
            tc.tile_pool(name="w1p", bufs=2) as w1p,        # [128, 8, 256] f16
            tc.tile_pool(name="atp", bufs=1) as atp,        # [128, 32, 1024] f16
            tc.tile_pool(name="w2p", bufs=2) as w2p,        # [128, 32, 128] f16
            tc.tile_pool(name="w1np", bufs=2) as w1np,      # [128, 4, 4, 512] f16
            tc.tile_pool(name="w2np", bufs=2) as w2np,      # [128, 4, 4, 512] f16
            tc.tile_pool(name="xpp", bufs=1) as xpp,
            tc.tile_pool(name="appp", bufs=1) as appp,      # [128, 8, 512] f16
            tc.tile_pool(name="atppp", bufs=1) as atppp,    # [128, 32, 128] f16
            tc.tile_pool(name="outp", bufs=3) as outp,      # [128, 512] f32
            tc.tile_pool(name="ypp", bufs=2) as ypp,        # [128, 512] f32
            tc.tile_pool(name="psum", bufs=2, space="PSUM") as psum,
        ):
            import contextlib

            identity = constp.tile([128, 128], f16)
            make_identity(nc, identity)

            loop_cm = (
                tc.For_i(0, loop_n, 1, hint_engines=(mybir.EngineType.PE,))
                if loop_n
                else contextlib.nullcontext()
            )
            with loop_cm:
              xp_sb = xpp.tile([128, KD, QPC * B], f16)
              nc.sync.dma_start(out=xp_sb[:], in_=xp_d[:])

              # ---------- shared branch ----------
              def shared_l1_block(blk):
                  at_b = atp.tile([128, MF, BLK], f16, name=f"at_{blk}", tag="at")
                  xq = [None, None]
                  for tch in range(2):
                      xq[tch] = xqp.tile([128, KD, 512], f16, name=f"xq_{blk}_{tch}", tag="xq")
                      nc.sync.dma_start(
                          out=xq[tch][:],
                          in_=xt_d[:, :, blk * BLK + tch * 512 : blk * BLK + (tch + 1) * 512],
                      )

                  def unit(g):
                      w1g = w1p.tile([128, KD, W1GW], f16, name=f"w1g_{blk}_{g}", tag="w1g")
                      nc.sync.dma_start(out=w1g[:], in_=w1_d[g])
                      for ml in range(2):
                          m = 2 * g + ml
                          for tch in range(2):
                              ps = psum.tile([128, 512], f32, tag="l1", bufs=2, name=f"psl1_{blk}_{m}_{tch}")
                              for k in range(KD):
                                  nc.tensor.matmul(
                                      ps[:],
                                      w1g[:, k, ml * 128 : (ml + 1) * 128],
                                      xq[tch][:, k, :],
                                      start=(k == 0),
                                      stop=(k == KD - 1),
                                  )
                              nc.scalar.activation(
                                  at_b[:, m, tch * 512 : (tch + 1) * 512], ps[:], GELU
                              )

                  return at_b, [lambda g=g: unit(g) for g in range(W1G)]

              def shared_l2_unit(blk, at_b, dq):
                  w2g = w2p.tile([128, MF, 128], f16, name=f"w2g_{blk}_{dq}", tag="w2g")
                  nc.sync.dma_start(out=w2g[:], in_=w2_d[dq])
                  for tch in range(2):
                      ps = psum.tile([128, 512], f32, tag="l2", bufs=2, name=f"psl2_{blk}_{dq}_{tch}")
                      for k in range(MF):
                          nc.tensor.matmul(
                              ps[:],
                              w2g[:, k, :],
                              at_b[:, k, tch * 512 : (tch + 1) * 512],
                              start=(k == 0),
                              stop=(k == MF - 1),
                          )
                      ot = outp.tile([128, 512], f32, name=f"ot_{blk}_{dq}_{tch}", tag="ot")
                      nc.vector.tensor_copy(ot[:], ps[:])
                      nc.sync.dma_start(
                          out=yst_d[
                              dq * 128 : (dq + 1) * 128,
                              blk * BLK + tch * 512 : blk * BLK + (tch + 1) * 512,
                          ],
                          in_=ot[:],
                      )

              # ---------- per-position branch (column-tiled over q) ----------
              pp_psum = {}

              # L1 chunk (nch, kh): 16 matmuls (4 q x 4 k) accumulating into
              # pp_psum[nch]; all four q's stream concurrently in distinct
              # PE column groups.
              def pp_l1_unit(app_all, nch, kh):
                  w1t = w1np.tile(
                      [128, QPC, KD // PPKH, PPW1], f16,
                      name=f"w1n_{nch}_{kh}", tag="w1n",
                  )
                  nc.sync.dma_start(out=w1t[:], in_=w1n_d[nch, kh])
                  if kh == 0:
                      pp_psum[nch] = psum.tile(
                          [128, PPW1], f32, tag="pp", bufs=2, name=f"pspp1_{nch}"
                      )
                  ps = pp_psum[nch]
                  for kk in range(KD // PPKH):
                      k = kh * (KD // PPKH) + kk
                      for q in range(QPC):
                          nc.tensor.matmul(
                              ps[32 * q : 32 * q + B, :],
                              xp_sb[:, k, q * B : (q + 1) * B],
                              w1t[:, q, kk, :],
                              start=(k == 0),
                              stop=(k == KD - 1),
                              tile_position=(0, 32 * q),
                              skip_group_check=True,
                          )
                  if kh == PPKH - 1:
                      nc.scalar.activation(app_all[:, nch, :], ps[:], GELU)

              # Transpose group tg (0..3): 8 PE transposes of [128,128]
              # app chunks into one f16 PSUM tile, then a single DVE copy
              # into atpp[:, tg*8:(tg+1)*8, :].
              def pp_tr_unit(app_all, atpp, tg):
                  pt = psum.tile([128, 8, 128], f16, tag="ptr", bufs=1, name=f"pstr_{tg}")
                  for j in range(8):
                      m = tg * 8 + j
                      nch, f = m // 4, m % 4
                      nc.tensor.transpose(
                          pt[:, j, :],
                          app_all[:, nch, f * 128 : (f + 1) * 128],
                          identity[:],
                      )
                  nc.vector.tensor_copy(atpp[:, tg * 8 : (tg + 1) * 8, :], pt[:])

              # L2 chunk (nch2, kg): 16 matmuls (4 q x 4 k) accumulating into
              # pp2_psum[nch2], column-tiled like L1.
              pp2_psum = {}

              def pp_l2_unit(atpp, nch2, kg):
                  w2t = w2np.tile(
                      [128, QPC, MF // PPKG, PPW2], f16,
                      name=f"w2n_{nch2}_{kg}", tag="w2n",
                  )
                  nc.sync.dma_start(out=w2t[:], in_=w2n_d[nch2, kg])
                  if kg == 0:
                      pp2_psum[nch2] = psum.tile(
                          [128, PPW2], f32, tag="pp", bufs=2, name=f"pspp2_{nch2}"
                      )
                  ps = pp2_psum[nch2]
                  for kk in range(MF // PPKG):
                      k = kg * (MF // PPKG) + kk
                      for q in range(QPC):
                          nc.tensor.matmul(
                              ps[32 * q : 32 * q + B, :],
                              atpp[:, k, 32 * q : 32 * q + B],
                              w2t[:, q, kk, :],
                              start=(k == 0),
                              stop=(k == MF - 1),
                              tile_position=(0, 32 * q),
                              skip_group_check=True,
                          )
                  if kg == PPKG - 1:
                      yt = ypp.tile([128, PPW2], f32, name=f"yt_{nch2}", tag="yt")
                      nc.vector.tensor_copy(yt[:], ps[:])
                      for q in range(QPC):
                          nc.sync.dma_start(
                              out=ytp_d[
                                  q * B : (q + 1) * B,
                                  nch2 * PPW2 : (nch2 + 1) * PPW2,
                              ],
                              in_=yt[32 * q : 32 * q + B, :],
                          )

              # ---------- emission schedule ----------
              side = []
              app_tiles = {}
              atpp_tiles = {}

              def mk_app():
                  app_tiles[0] = appp.tile([128, PPN1, PPW1], f16, name="app", tag="app")

              def mk_atpp():
                  atpp_tiles[0] = atppp.tile([128, MF, 128], f16, name="atpp", tag="atpp")

              side.append(mk_app)
              for nch in range(PPN1):
                  for kh in range(PPKH):
                      side.append(lambda nch=nch, kh=kh: pp_l1_unit(app_tiles[0], nch, kh))
              side.append(mk_atpp)
              for tg in range(4):
                  side.append(lambda tg=tg: pp_tr_unit(app_tiles[0], atpp_tiles[0], tg))
              for nch2 in range(PPN2):
                  for kg in range(PPKG):
                      side.append(lambda nch2=nch2, kg=kg: pp_l2_unit(atpp_tiles[0], nch2, kg))

              side_i = 0

              def emit_side(n):
                  nonlocal side_i
                  budget = n
                  while side_i < len(side) and budget > 0:
                      side[side_i]()
                      side_i += 1
                      budget -= 1

              if parts == "pp":
                  emit_side(len(side))
              else:
                  # Side list length: 1 + 16 + 1 + 4 + 16 = 38 units.
                  # Shared units: 2 blocks * (16 L1 + 8 L2) = 48.
                  do_side = parts == "all"
                  for blk in range(NBLK):
                      at_b, l1_units = shared_l1_block(blk)
                      for g, u in enumerate(l1_units):
                          u()
                          if do_side:
                              emit_side(1)
                      for dq in range(NQ):
                          shared_l2_unit(blk, at_b, dq)
                          if do_side:
                              emit_side(1)
                  if do_side:
                      emit_side(len(side))

    nc.compile()
    _CACHE[key] = nc
    return nc


def pack_inputs(x, W1S, W2S, W1NS, W2NS):
    """Build the 8 per-core input maps (numpy, host-side layout packing)."""
    x = np.asarray(x, dtype=np.float32)
    W1S = np.asarray(W1S, dtype=np.float32)
    W2S = np.asarray(W2S, dtype=np.float32)
    W1NS = np.asarray(W1NS, dtype=np.float32)
    W2NS = np.asarray(W2NS, dtype=np.float32)

    # Shared weights: identical on every core.
    w1_pk = np.ascontiguousarray(
        W1S.reshape(KD, 128, W1G, W1GW).transpose(2, 1, 0, 3).astype(np.float16)
    )
    w2_pk = np.ascontiguousarray(
        W2S.reshape(MF, 128, NQ, 128).transpose(2, 1, 0, 3).astype(np.float16)
    )

    in_maps = []
    for c in range(NCORES):
        xs = np.zeros((TPAD, D), dtype=np.float32)
        xs[:S] = x[c, :S]
        xt = np.ascontiguousarray(
            xs.T.reshape(KD, 128, TPAD).transpose(1, 0, 2).astype(np.float16)
        )

        # w1n: [QPC, D, F] -> [PPN1, PPKH, 128, QPC, KD//PPKH, PPW1]
        w1n = np.ascontiguousarray(
            W1NS[QPC * c : QPC * (c + 1)]
            .reshape(QPC, PPKH, KD // PPKH, 128, PPN1, PPW1)
            .transpose(4, 1, 3, 0, 2, 5)
            .astype(np.float16)
        )
        # w2n: [QPC, F, D] -> [PPN2, PPKG, 128, QPC, MF//PPKG, PPW2]
        w2n = np.ascontiguousarray(
            W2NS[QPC * c : QPC * (c + 1)]
            .reshape(QPC, PPKG, MF // PPKG, 128, PPN2, PPW2)
            .transpose(4, 1, 3, 0, 2, 5)
            .astype(np.float16)
        )
        xpos = x[:, S + QPC * c : S + QPC * (c + 1), :]          # [B, QPC, D]
        xp = np.ascontiguousarray(
            xpos.transpose(2, 1, 0)
            .reshape(KD, 128, QPC * B)
            .transpose(1, 0, 2)
            .astype(np.float16)
        )
        in_maps.append(
            {"xt": xt, "w1": w1_pk, "w2": w2_pk, "w1n": w1n, "w2n": w2n, "xp": xp}
        )
    return in_maps


def unpack_outputs(results):
    """Assemble the full [B, T, D] output from the 8 per-core result maps."""
    out = np.empty((B, T, D), dtype=np.float32)
    for c in range(NCORES):
        yst = results[c]["yst"]          # [D, TPAD]
        ytp = results[c]["ytp"]          # [B*QPC, D]
        out[c, :S, :] = yst[:, :S].T
        for q in range(QPC):
            out[:, S + QPC * c + q, :] = ytp[q * B : (q + 1) * B, :]
    return out


def kernel(x, W1S, W2S, W1NS, W2NS):
    from concourse.bass_utils import run_bass_kernel_spmd

    nc = _build_nc()
    in_maps = pack_inputs(x, W1S, W2S, W1NS, W2NS)
    res = run_bass_kernel_spmd(nc, in_maps, core_ids=list(range(NCORES)))
    return unpack_outputs(res.results)
